# revision 23
# baseline (speedup 1.0000x reference)
"""Contrastive cosine-similarity softmax-CE loss on 8 trn2 NeuronCores.

reference math:
    n1 = f1 / max(||f1||, eps);  n2 = f2 / max(||f2||, eps)
    logits = (n1 @ n2.T) / TEMP                      # [8192, 8192]
    loss = mean_i( logsumexp_j(logits[i, :]) - logits[i, i] )

sharding: f1 rows data-parallel across 8 cores (1024 rows each); f2
replicated (each core streams all of f2 from its HBM copy).  Per-core
output is the vector of per-row (lse - l_ii); host averages.

Device-side algorithm per core (all SPMD-uniform, no collectives):
  - logits are never max-subtracted: |logit| <= 1/0.07 = 14.29 by
    Cauchy-Schwarz, so exp() stays within fp32 range (max e^14.3=1.6e6,
    row-sum <= 1.3e10 << fp32 max).  Single-pass softmax.
  - the eps clamp of the reference (||f|| >= 1e-8) is a mathematical
    no-op for these inputs (||f||^2 ~ chi2(768), concentrated at ~768)
    and is skipped.
  - f1 is NOT normalized before the GEMM; inv-norm/TEMP rides in as the
    per-partition `scale` operand of the fused Exp activation.
  - f2 IS normalized pre-GEMM (its inv-norm varies along the free dim).
    Sum-of-squares per f2-row is computed with a ones[128,128] matmul on
    the tensor engine (which also broadcasts the result across all 128
    partitions for free); inv-norm = Exp(-0.5*Ln(x)) so the whole kernel
    uses the single natural_log_exp ACT table set (Rsqrt activation is
    banned for accuracy in this stack).
  - fp32->bf16 casts of the GEMM operands happen inside the SWDGE DMA.
"""

import sys

for _p in ("/opt/trn_rl_repo",):
    if _p not in sys.path:
        sys.path.insert(0, _p)

from contextlib import ExitStack

import numpy as np

import concourse.bass as bass
import concourse.tile as tile
from concourse import mybir

FP32 = mybir.dt.float32
BF16 = mybir.dt.bfloat16
AF = mybir.ActivationFunctionType
ALU = mybir.AluOpType
AX = mybir.AxisListType

N = 8192        # rows of f1/f2
D = 768         # feature dim
NCORES = 8
MC = N // NCORES        # f1 rows per core (1024)
KT = D // 128           # contraction k-chunks (6)
MT = MC // 128          # f1 row tiles per core (8)
PAIR = 1024             # f2 rows processed per outer step
NPAIR = N // PAIR       # 8
TEMP = 0.07
LOG_INV_TEMP = float(-np.log(TEMP))


_WAIT_SPLIT_SKIP = (
    "InstEventSemaphore",
    "InstHalt",
)


def _split_excess_waits(nc: bass.Bass, cap: int = 1) -> None:
    """Hoist per-instruction sync waits beyond `cap` into standalone
    InstEventSemaphore instructions on the same engine.

    The 64-byte TPB instruction encodings carry very few embedded wait
    slots (one for TensorTensor, two for the DMA pseudo-ops, ...) and
    walrus codegen hard-fails on overflow ("Too many sync wait commands").
    Tile's scheduler happily attaches more, so we split them here.
    """
    n = 0
    for bb in nc.main_func.blocks:
        new_list = []
        for inst in bb.instructions:
            si = inst.sync_info
            ow = list(si.on_wait) if si is not None and si.on_wait else []
            if len(ow) > cap and type(inst).__name__ not in _WAIT_SPLIT_SKIP:
                excess, keep = ow[:-cap], ow[-cap:]
                for w in excess:
                    n += 1
                    ev = mybir.InstEventSemaphore(
                        name=f"I-waitsplit-{n}",
                        engine=inst.engine,
                        ins=[],
                        outs=[],
                        sync_info=mybir.SyncInfo(on_wait=[w], on_update=[]),
                    )
                    nc.register_instruction(ev)
                    new_list.append(ev)
                si.on_wait = keep
            new_list.append(inst)
        bb.instructions[:] = new_list


def build_program() -> bass.Bass:
    nc = bass.Bass()
    f1n = nc.declare_dram_parameter("f1n", [MC, D], FP32, isOutput=False)
    f1t = nc.declare_dram_parameter("f1t", [D, MC], FP32, isOutput=False)
    f2t = nc.declare_dram_parameter("f2t", [D, N], FP32, isOutput=False)
    f2dn = nc.declare_dram_parameter("f2dn", [MC, D], FP32, isOutput=False)
    out = nc.declare_dram_parameter("out", [128, MT], FP32, isOutput=True)

    with tile.TileContext(nc) as tc, ExitStack() as ctx:
        singles = ctx.enter_context(tc.tile_pool(name="singles", bufs=1))

        ones = singles.tile([128, 128], BF16, tag="ones", name="ones")
        nc.any.memset(ones[:], 1.0)
        lbias = singles.tile([128, 1], FP32, tag="lbias", name="lbias")
        nc.any.memset(lbias[:], LOG_INV_TEMP)
        hbias = singles.tile([128, 1], FP32, tag="hbias", name="hbias")
        nc.any.memset(hbias[:], float(np.log(0.5)))

        # resident bf16 operands: f1^T (raw) and normalized f2^T
        n1t = [
            singles.tile([128, MC], BF16, tag=f"n1t{k}", name=f"n1t{k}")
            for k in range(KT)
        ]
        invn1T = singles.tile([128, MT], FP32, tag="invn1T", name="invn1T")
        dvals = singles.tile([128, MT], FP32, tag="dvals", name="dvals")
        spart = singles.tile([128, MT * NPAIR], FP32, tag="spart", name="spart")

        # ---- P0: f1-side prep (natural layout; rows on partitions) ----
        # Every DMA writes a FRESH slice of a big staging tile — no buffer
        # recycling, so no WAW/WAR sync waits land on DMA instructions
        # (the 64B DMA pseudo-instruction encodes at most 2 waits).
        with tc.tile_pool(name="p0", bufs=1) as p0, tc.tile_pool(
            name="p0s", bufs=1
        ) as p0s:
            ss1 = p0s.tile([128, MT], FP32, tag="ss1", name="ss1")
            ss2 = p0s.tile([128, MT], FP32, tag="ss2", name="ss2")
            sssum = p0s.tile([128, MT], FP32, tag="sssum", name="sssum")
            draw = p0s.tile([128, MT], FP32, tag="draw", name="draw")
            a_all = p0.tile([128, MT, D], FP32, tag="a_all", name="a_all")
            b_all = p0.tile([128, MT, D], FP32, tag="b_all", name="b_all")
            ab_all = p0.tile([128, MT, D], FP32, tag="ab_all", name="ab_all")
            # Pool-engine absorbers: the first touch of a fresh pool region
            # carries the released-zone deps; putting that on gpsimd memsets
            # keeps the (wait-limited) DMA instructions clean.
            nc.gpsimd.memset(a_all[:], 0.0)
            nc.gpsimd.memset(b_all[:], 0.0)
            nc.gpsimd.memset(ab_all[:], 0.0)
            for m in range(MT):
                nc.gpsimd.dma_start(a_all[:, m, :], f1n[m * 128 : (m + 1) * 128, :])
                nc.gpsimd.dma_start(b_all[:, m, :], f2dn[m * 128 : (m + 1) * 128, :])
                # a+b via accumulating SWDGE DMA (re-reads from DRAM); a
                # TensorTensor add would exceed the single TT wait slot.
                nc.gpsimd.dma_start(ab_all[:, m, :], f1n[m * 128 : (m + 1) * 128, :])
                nc.gpsimd.dma_start(
                    ab_all[:, m, :],
                    f2dn[m * 128 : (m + 1) * 128, :],
                    accum_op=ALU.add,
                )
            for m in range(MT):
                # sum-of-squares per row via fused Square + accumulate
                sqa = p0.tile([128, D], BF16, tag="sqa", name="sqa", bufs=2)
                nc.scalar.activation(
                    sqa[:], a_all[:, m, :], AF.Square, accum_out=ss1[:, m : m + 1]
                )
                sqb = p0.tile([128, D], BF16, tag="sqb", name="sqb", bufs=2)
                nc.scalar.activation(
                    sqb[:], b_all[:, m, :], AF.Square, accum_out=ss2[:, m : m + 1]
                )
                sqc = p0.tile([128, D], BF16, tag="sqc", name="sqc", bufs=2)
                nc.scalar.activation(
                    sqc[:], ab_all[:, m, :], AF.Square, accum_out=sssum[:, m : m + 1]
                )
            # invn1T = exp(-0.5*ln(ss1) + ln(1/T)) = 1/(T*||f1_i||)
            t1 = p0s.tile([128, MT], FP32, tag="t1", name="t1")
            nc.scalar.activation(t1[:], ss1[:], AF.Ln)
            nc.scalar.activation(
                invn1T[:], t1[:], AF.Exp, scale=-0.5, bias=lbias[:]
            )
            # invn2d = 1/||f2_i|| for the diagonal block
            # invn2d carries the polarization 0.5: 0.5/||f2d_i||
            t2 = p0s.tile([128, MT], FP32, tag="t2", name="t2")
            nc.scalar.activation(t2[:], ss2[:], AF.Ln)
            invn2d = p0s.tile([128, MT], FP32, tag="invn2d", name="invn2d")
            nc.scalar.activation(invn2d[:], t2[:], AF.Exp, scale=-0.5, bias=hbias[:])
            # draw = sssum - ss1 - ss2 = 2*<f1_i, f2_i>
            # (copy first: absorbs the cross-engine wait so each TensorTensor
            # keeps <=1 sync wait — the 64B TT encoding has one wait slot)
            t4 = p0s.tile([128, MT], FP32, tag="t4", name="t4")
            nc.vector.tensor_copy(t4[:], sssum[:])
            nc.vector.tensor_sub(t4[:], t4[:], ss1[:])
            nc.vector.tensor_sub(draw[:], t4[:], ss2[:])
            # dvals = draw * invn1T * invn2d   (logit value on the diagonal)
            t3 = p0s.tile([128, MT], FP32, tag="t3", name="t3")
            nc.vector.tensor_mul(t3[:], draw[:], invn1T[:])
            nc.vector.tensor_mul(dvals[:], t3[:], invn2d[:])

        # f1^T cast to bf16 during DMA (SWDGE cast)
        for k in range(KT):
            nc.gpsimd.dma_start(n1t[k][:], f1t[k * 128 : (k + 1) * 128, :])

        # ---- P1+P2 fused: stream f2^T, normalize, GEMM, fused exp-sum ----
        # The cast-DMA writes straight into fresh columns of the resident
        # n2st tiles (no staging buffer, so the DMA carries no waits);
        # normalization then happens in place.
        f2t_r = f2t[:, :].rearrange("(k p) n -> p k n", p=128)  # [128, KT, N]
        with tc.tile_pool(name="n2p", bufs=1) as n2p, tc.tile_pool(
            name="wk", bufs=2
        ) as wp, tc.tile_pool(name="pss", bufs=2, space="PSUM") as pp, tc.tile_pool(
            name="psl", bufs=2, space="PSUM"
        ) as pl, tc.tile_pool(name="expp", bufs=3) as ep:
            n2st = [
                n2p.tile([128, N], BF16, tag=f"n2st{k}", name=f"n2st{k}")
                for k in range(KT)
            ]
            for k in range(KT):
                nc.gpsimd.memset(n2st[k][:], 0.0)
            for pair in range(NPAIR):
                c0 = pair * PAIR
                # load (and cast) raw f2^T chunk into n2st columns
                for k in range(KT):
                    nc.gpsimd.dma_start(
                        n2st[k][:, c0 : c0 + PAIR], f2t_r[:, k, c0 : c0 + PAIR]
                    )
                # pre-touch: a copy absorbs the DMA waits so the squares'
                # TensorTensor ops carry at most one sync wait each
                tch = wp.tile([128, 64], BF16, tag="tch", name="tch")
                nc.vector.tensor_copy(tch[:], n2st[0][:, c0 : c0 + 64])
                sq = wp.tile([128, KT, PAIR], BF16, tag="sq", name="sq")
                # write-touch: absorbs the WAR wait on sq's recycled slot
                nc.vector.tensor_copy(sq[:, 0, 0:64], tch[:])
                for k in range(KT):
                    nc.vector.tensor_mul(
                        sq[:, k, :],
                        n2st[k][:, c0 : c0 + PAIR],
                        n2st[k][:, c0 : c0 + PAIR],
                    )
                # per-f2-row sum of squares, broadcast to all 128 partitions
                ss = pp.tile([128, PAIR], FP32, tag="ss", name="ss")
                for k in range(KT):
                    for h in range(2):
                        nc.tensor.matmul(
                            ss[:, h * 512 : (h + 1) * 512],
                            ones[:],
                            sq[:, k, h * 512 : (h + 1) * 512],
                            start=(k == 0),
                            stop=(k == KT - 1),
                        )
                lntmp = wp.tile([128, PAIR], FP32, tag="lntmp", name="lntmp")
                nc.scalar.activation(lntmp[:], ss[:], AF.Ln)
                invn2 = wp.tile([128, PAIR], BF16, tag="invn2", name="invn2")
                nc.scalar.activation(invn2[:], lntmp[:], AF.Exp, scale=-0.5)
                # normalize the resident chunk in place
                for k in range(KT):
                    nc.vector.tensor_mul(
                        n2st[k][:, c0 : c0 + PAIR],
                        n2st[k][:, c0 : c0 + PAIR],
                        invn2[:],
                    )
                # main GEMM + fused exp/row-sum for every f1 row-tile
                for m in range(MT):
                    pslog = pl.tile([128, PAIR], FP32, tag="pslog", name="pslog")
                    for k in range(KT):
                        for h in range(2):
                            nc.tensor.matmul(
                                pslog[:, h * 512 : (h + 1) * 512],
                                n1t[k][:, m * 128 : (m + 1) * 128],
                                n2st[k][:, c0 + h * 512 : c0 + (h + 1) * 512],
                                start=(k == 0),
                                stop=(k == KT - 1),
                            )
                    eb = ep.tile([128, PAIR], BF16, tag="eb", name="eb")
                    col = m * NPAIR + pair
                    nc.scalar.activation(
                        eb[:],
                        pslog[:],
                        AF.Exp,
                        scale=invn1T[:, m : m + 1],
                        accum_out=spart[:, col : col + 1],
                    )

            # ---- P3: finalize ----
            S = ep.tile([128, MT], FP32, tag="S", name="S")
            nc.vector.reduce_sum(
                S[:], spart[:].rearrange("p (m q) -> p m q", q=NPAIR), axis=AX.X
            )
            lse = ep.tile([128, MT], FP32, tag="lse", name="lse")
            nc.scalar.activation(lse[:], S[:], AF.Ln)
            res = ep.tile([128, MT], FP32, tag="res", name="res")
            nc.vector.tensor_sub(res[:], lse[:], dvals[:])
            nc.gpsimd.dma_start(out[:, :], res[:])

    _split_excess_waits(nc)
    return nc


def make_in_maps(f1: np.ndarray, f2: np.ndarray) -> list[dict[str, np.ndarray]]:
    f1 = np.ascontiguousarray(np.asarray(f1, dtype=np.float32))
    f2 = np.ascontiguousarray(np.asarray(f2, dtype=np.float32))
    assert f1.shape == (N, D) and f2.shape == (N, D)
    f2t = np.ascontiguousarray(f2.T)  # [D, N], shared by all cores
    in_maps = []
    for c in range(NCORES):
        f1c = np.ascontiguousarray(f1[c * MC : (c + 1) * MC])
        in_maps.append(
            {
                "f1n": f1c,
                "f1t": np.ascontiguousarray(f1c.T),
                "f2t": f2t,
                "f2dn": np.ascontiguousarray(f2[c * MC : (c + 1) * MC]),
            }
        )
    return in_maps


def combine_outputs(outs: list[np.ndarray]) -> np.float32:
    total = 0.0
    for o in outs:
        total += float(np.sum(np.asarray(o, dtype=np.float64)))
    return np.float32(total / float(N))


def run(f1: np.ndarray, f2: np.ndarray, trace: bool = False):
    from concourse.bass_utils import run_bass_kernel_spmd

    nc = build_program()
    in_maps = make_in_maps(f1, f2)
    r = run_bass_kernel_spmd(nc, in_maps, core_ids=list(range(NCORES)), trace=trace)
    outs = [m["out"] for m in r.results]
    return combine_outputs(outs), r


def kernel(f1: np.ndarray, f2: np.ndarray) -> np.ndarray:
    loss, _ = run(f1, f2, trace=False)
    return loss


if __name__ == "__main__":
    f1 = np.random.randn(N, D).astype(np.float32)
    f2 = np.random.randn(N, D).astype(np.float32)
    print(kernel(f1, f2))


# revision 24
# speedup vs baseline: 1.0882x; 1.0882x over previous
"""Contrastive cosine-similarity softmax-CE loss on 8 trn2 NeuronCores.

reference math:
    n1 = f1 / max(||f1||, eps);  n2 = f2 / max(||f2||, eps)
    logits = (n1 @ n2.T) / TEMP                      # [8192, 8192]
    loss = mean_i( logsumexp_j(logits[i, :]) - logits[i, i] )

sharding: f1 rows data-parallel across 8 cores (1024 rows each); f2
replicated (each core streams all of f2 from its HBM copy).  Per-core
output is the vector of per-row (lse - l_ii); host averages.

Device-side algorithm per core (all SPMD-uniform, no collectives):
  - logits are never max-subtracted: |logit| <= 1/0.07 = 14.29 by
    Cauchy-Schwarz, so exp() stays within fp32 range (max e^14.3=1.6e6,
    row-sum <= 1.3e10 << fp32 max).  Single-pass softmax.
  - the eps clamp of the reference (||f|| >= 1e-8) is a mathematical
    no-op for these inputs (||f||^2 ~ chi2(768), concentrated at ~768)
    and is skipped.
  - f1 is NOT normalized before the GEMM; inv-norm/TEMP rides in as the
    per-partition `scale` operand of the fused Exp activation.
  - f2 IS normalized pre-GEMM (its inv-norm varies along the free dim).
    Sum-of-squares per f2-row is computed with a ones[128,128] matmul on
    the tensor engine (which also broadcasts the result across all 128
    partitions for free); inv-norm = Exp(-0.5*Ln(x)) so the whole kernel
    uses the single natural_log_exp ACT table set (Rsqrt activation is
    banned for accuracy in this stack).
  - fp32->bf16 casts of the GEMM operands happen inside the SWDGE DMA.
"""

import sys

for _p in ("/opt/trn_rl_repo",):
    if _p not in sys.path:
        sys.path.insert(0, _p)

from contextlib import ExitStack

import numpy as np

import concourse.bass as bass
import concourse.tile as tile
from concourse import mybir

FP32 = mybir.dt.float32
BF16 = mybir.dt.bfloat16
AF = mybir.ActivationFunctionType
ALU = mybir.AluOpType
AX = mybir.AxisListType

N = 8192        # rows of f1/f2
D = 768         # feature dim
NCORES = 8
MC = N // NCORES        # f1 rows per core (1024)
KT = D // 128           # contraction k-chunks (6)
MT = MC // 128          # f1 row tiles per core (8)
PAIR = 1024             # f2 rows processed per outer step
NPAIR = N // PAIR       # 8
TEMP = 0.07
LOG_INV_TEMP = float(-np.log(TEMP))


_WAIT_SPLIT_SKIP = (
    "InstEventSemaphore",
    "InstHalt",
)


def _split_excess_waits(nc: bass.Bass, cap: int = 1) -> None:
    """Hoist per-instruction sync waits beyond `cap` into standalone
    InstEventSemaphore instructions on the same engine.

    The 64-byte TPB instruction encodings carry very few embedded wait
    slots (one for TensorTensor, two for the DMA pseudo-ops, ...) and
    walrus codegen hard-fails on overflow ("Too many sync wait commands").
    Tile's scheduler happily attaches more, so we split them here.
    """
    n = 0
    for bb in nc.main_func.blocks:
        new_list = []
        for inst in bb.instructions:
            si = inst.sync_info
            ow = list(si.on_wait) if si is not None and si.on_wait else []
            if len(ow) > cap and type(inst).__name__ not in _WAIT_SPLIT_SKIP:
                excess, keep = ow[:-cap], ow[-cap:]
                for w in excess:
                    n += 1
                    ev = mybir.InstEventSemaphore(
                        name=f"I-waitsplit-{n}",
                        engine=inst.engine,
                        ins=[],
                        outs=[],
                        sync_info=mybir.SyncInfo(on_wait=[w], on_update=[]),
                    )
                    nc.register_instruction(ev)
                    new_list.append(ev)
                si.on_wait = keep
            new_list.append(inst)
        bb.instructions[:] = new_list


def build_program() -> bass.Bass:
    nc = bass.Bass()
    f1n = nc.declare_dram_parameter("f1n", [MC, D], FP32, isOutput=False)
    f1t = nc.declare_dram_parameter("f1t", [D, MC], FP32, isOutput=False)
    f2t = nc.declare_dram_parameter("f2t", [D, N], FP32, isOutput=False)
    f2dn = nc.declare_dram_parameter("f2dn", [MC, D], FP32, isOutput=False)
    out = nc.declare_dram_parameter("out", [128, MT], FP32, isOutput=True)

    with tile.TileContext(nc, pool_alloc_mode="queue") as tc, ExitStack() as ctx:
        singles = ctx.enter_context(tc.tile_pool(name="singles", bufs=1))

        ones = singles.tile([128, 128], BF16, tag="ones", name="ones")
        nc.any.memset(ones[:], 1.0)
        lbias = singles.tile([128, 1], FP32, tag="lbias", name="lbias")
        nc.any.memset(lbias[:], LOG_INV_TEMP)
        hbias = singles.tile([128, 1], FP32, tag="hbias", name="hbias")
        nc.any.memset(hbias[:], float(np.log(0.5)))

        # resident bf16 operands: f1^T (raw) and normalized f2^T
        n1t = [
            singles.tile([128, MC], BF16, tag=f"n1t{k}", name=f"n1t{k}")
            for k in range(KT)
        ]
        invn1T = singles.tile([128, MT], FP32, tag="invn1T", name="invn1T")
        dvals = singles.tile([128, MT], FP32, tag="dvals", name="dvals")
        spart = singles.tile([128, MT * NPAIR], FP32, tag="spart", name="spart")

        n2p = ctx.enter_context(tc.tile_pool(name="n2p", bufs=1))
        n2st = [
            n2p.tile([128, N], BF16, tag=f"n2st{k}", name=f"n2st{k}")
            for k in range(KT)
        ]

        # ---- hoisted loads: issue every SWDGE stream DMA up front ----
        # f1^T cast to bf16 during DMA (SWDGE cast); needed by the first MMs
        for k in range(KT):
            nc.gpsimd.dma_start(n1t[k][:], f1t[k * 128 : (k + 1) * 128, :])
        # f2^T cast-loads straight into fresh columns of the resident n2st
        f2t_r = f2t[:, :].rearrange("(k p) n -> p k n", p=128)  # [128, KT, N]
        for pair in range(NPAIR):
            c0 = pair * PAIR
            for k in range(KT):
                nc.gpsimd.dma_start(
                    n2st[k][:, c0 : c0 + PAIR], f2t_r[:, k, c0 : c0 + PAIR]
                )

        # ---- P0: f1-side prep (natural layout), concurrent with the stream.
        # a/b ride the HWDGE (sync) queue so the Pool queue stays dedicated
        # to the f2 stream; the a+b polarization load needs SWDGE accum.
        with tc.tile_pool(name="p0", bufs=1) as p0, tc.tile_pool(
            name="p0s", bufs=1
        ) as p0s:
            ss1 = p0s.tile([128, MT], FP32, tag="ss1", name="ss1")
            ss2 = p0s.tile([128, MT], FP32, tag="ss2", name="ss2")
            sssum = p0s.tile([128, MT], FP32, tag="sssum", name="sssum")
            draw = p0s.tile([128, MT], FP32, tag="draw", name="draw")
            a_all = p0.tile([128, MT, D], FP32, tag="a_all", name="a_all")
            b_all = p0.tile([128, MT, D], FP32, tag="b_all", name="b_all")
            ab_all = p0.tile([128, MT, D], FP32, tag="ab_all", name="ab_all")
            for m in range(MT):
                nc.sync.dma_start(a_all[:, m, :], f1n[m * 128 : (m + 1) * 128, :])
                nc.sync.dma_start(b_all[:, m, :], f2dn[m * 128 : (m + 1) * 128, :])
                nc.gpsimd.dma_start(ab_all[:, m, :], f1n[m * 128 : (m + 1) * 128, :])
                nc.gpsimd.dma_start(
                    ab_all[:, m, :],
                    f2dn[m * 128 : (m + 1) * 128, :],
                    accum_op=ALU.add,
                )
            for m in range(MT):
                sqa = p0.tile([128, D], BF16, tag="sqa", name="sqa", bufs=2)
                nc.scalar.activation(
                    sqa[:], a_all[:, m, :], AF.Square, accum_out=ss1[:, m : m + 1]
                )
                sqb = p0.tile([128, D], BF16, tag="sqb", name="sqb", bufs=2)
                nc.scalar.activation(
                    sqb[:], b_all[:, m, :], AF.Square, accum_out=ss2[:, m : m + 1]
                )
                sqc = p0.tile([128, D], BF16, tag="sqc", name="sqc", bufs=2)
                nc.scalar.activation(
                    sqc[:], ab_all[:, m, :], AF.Square, accum_out=sssum[:, m : m + 1]
                )
            # invn1T = exp(-0.5*ln(ss1) + ln(1/T)) = 1/(T*||f1_i||)
            t1 = p0s.tile([128, MT], FP32, tag="t1", name="t1")
            nc.scalar.activation(t1[:], ss1[:], AF.Ln)
            nc.scalar.activation(invn1T[:], t1[:], AF.Exp, scale=-0.5, bias=lbias[:])
            # invn2d carries the polarization 0.5: 0.5/||f2d_i||
            t2 = p0s.tile([128, MT], FP32, tag="t2", name="t2")
            nc.scalar.activation(t2[:], ss2[:], AF.Ln)
            invn2d = p0s.tile([128, MT], FP32, tag="invn2d", name="invn2d")
            nc.scalar.activation(invn2d[:], t2[:], AF.Exp, scale=-0.5, bias=hbias[:])
            # draw = sssum - ss1 - ss2 = 2*<f1_i, f2_i>
            t4 = p0s.tile([128, MT], FP32, tag="t4", name="t4")
            nc.vector.tensor_sub(t4[:], sssum[:], ss1[:])
            nc.vector.tensor_sub(draw[:], t4[:], ss2[:])
            # dvals = draw * invn1T * invn2d   (logit value on the diagonal)
            t3 = p0s.tile([128, MT], FP32, tag="t3", name="t3")
            nc.vector.tensor_mul(t3[:], draw[:], invn1T[:])
            nc.vector.tensor_mul(dvals[:], t3[:], invn2d[:])

        # ---- pair pipeline: normalize chunk, GEMM, fused exp/row-sum ----
        with tc.tile_pool(name="wk", bufs=2) as wp, tc.tile_pool(
            name="pss", bufs=2, space="PSUM"
        ) as pp, tc.tile_pool(name="psl", bufs=2, space="PSUM") as pl, tc.tile_pool(
            name="expp", bufs=3
        ) as ep:
            for pair in range(NPAIR):
                c0 = pair * PAIR
                sq = wp.tile([128, KT, PAIR], BF16, tag="sq", name="sq")
                for k in range(KT):
                    nc.vector.tensor_mul(
                        sq[:, k, :],
                        n2st[k][:, c0 : c0 + PAIR],
                        n2st[k][:, c0 : c0 + PAIR],
                    )
                # per-f2-row sum of squares, broadcast to all 128 partitions
                ss = pp.tile([128, PAIR], FP32, tag="ss", name="ss")
                for k in range(KT):
                    for h in range(2):
                        nc.tensor.matmul(
                            ss[:, h * 512 : (h + 1) * 512],
                            ones[:],
                            sq[:, k, h * 512 : (h + 1) * 512],
                            start=(k == 0),
                            stop=(k == KT - 1),
                        )
                lntmp = wp.tile([128, PAIR], FP32, tag="lntmp", name="lntmp")
                nc.scalar.activation(lntmp[:], ss[:], AF.Ln)
                invn2 = wp.tile([128, PAIR], BF16, tag="invn2", name="invn2")
                nc.scalar.activation(invn2[:], lntmp[:], AF.Exp, scale=-0.5)
                # normalize the resident chunk in place
                for k in range(KT):
                    nc.vector.tensor_mul(
                        n2st[k][:, c0 : c0 + PAIR],
                        n2st[k][:, c0 : c0 + PAIR],
                        invn2[:],
                    )
                # main GEMM + fused exp/row-sum for every f1 row-tile
                for m in range(MT):
                    pslog = pl.tile([128, PAIR], FP32, tag="pslog", name="pslog")
                    for k in range(KT):
                        for h in range(2):
                            nc.tensor.matmul(
                                pslog[:, h * 512 : (h + 1) * 512],
                                n1t[k][:, m * 128 : (m + 1) * 128],
                                n2st[k][:, c0 + h * 512 : c0 + (h + 1) * 512],
                                start=(k == 0),
                                stop=(k == KT - 1),
                            )
                    eb = ep.tile([128, PAIR], BF16, tag="eb", name="eb")
                    col = m * NPAIR + pair
                    nc.scalar.activation(
                        eb[:],
                        pslog[:],
                        AF.Exp,
                        scale=invn1T[:, m : m + 1],
                        accum_out=spart[:, col : col + 1],
                    )

            # ---- finalize ----
            S = ep.tile([128, MT], FP32, tag="S", name="S")
            nc.vector.reduce_sum(
                S[:], spart[:].rearrange("p (m q) -> p m q", q=NPAIR), axis=AX.X
            )
            lse = ep.tile([128, MT], FP32, tag="lse", name="lse")
            nc.scalar.activation(lse[:], S[:], AF.Ln)
            res = ep.tile([128, MT], FP32, tag="res", name="res")
            nc.vector.tensor_sub(res[:], lse[:], dvals[:])
            nc.sync.dma_start(out[:, :], res[:])

    _split_excess_waits(nc)
    return nc


def make_in_maps(f1: np.ndarray, f2: np.ndarray) -> list[dict[str, np.ndarray]]:
    f1 = np.ascontiguousarray(np.asarray(f1, dtype=np.float32))
    f2 = np.ascontiguousarray(np.asarray(f2, dtype=np.float32))
    assert f1.shape == (N, D) and f2.shape == (N, D)
    f2t = np.ascontiguousarray(f2.T)  # [D, N], shared by all cores
    in_maps = []
    for c in range(NCORES):
        f1c = np.ascontiguousarray(f1[c * MC : (c + 1) * MC])
        in_maps.append(
            {
                "f1n": f1c,
                "f1t": np.ascontiguousarray(f1c.T),
                "f2t": f2t,
                "f2dn": np.ascontiguousarray(f2[c * MC : (c + 1) * MC]),
            }
        )
    return in_maps


def combine_outputs(outs: list[np.ndarray]) -> np.float32:
    total = 0.0
    for o in outs:
        total += float(np.sum(np.asarray(o, dtype=np.float64)))
    return np.float32(total / float(N))


def run(f1: np.ndarray, f2: np.ndarray, trace: bool = False):
    from concourse.bass_utils import run_bass_kernel_spmd

    nc = build_program()
    in_maps = make_in_maps(f1, f2)
    r = run_bass_kernel_spmd(nc, in_maps, core_ids=list(range(NCORES)), trace=trace)
    outs = [m["out"] for m in r.results]
    return combine_outputs(outs), r


def kernel(f1: np.ndarray, f2: np.ndarray) -> np.ndarray:
    loss, _ = run(f1, f2, trace=False)
    return loss


if __name__ == "__main__":
    f1 = np.random.randn(N, D).astype(np.float32)
    f2 = np.random.randn(N, D).astype(np.float32)
    print(kernel(f1, f2))


# revision 26
# speedup vs baseline: 1.1906x; 1.0941x over previous
"""Contrastive cosine-similarity softmax-CE loss on 8 trn2 NeuronCores.

reference math:
    n1 = f1 / max(||f1||, eps);  n2 = f2 / max(||f2||, eps)
    logits = (n1 @ n2.T) / TEMP                      # [8192, 8192]
    loss = mean_i( logsumexp_j(logits[i, :]) - logits[i, i] )

sharding: f1 rows data-parallel across 8 cores (1024 rows each); f2
replicated (each core streams all of f2 from its HBM copy).  Per-core
output is the vector of per-row (lse - l_ii); host averages.

Device-side algorithm per core (all SPMD-uniform, no collectives):
  - logits are never max-subtracted: |logit| <= 1/0.07 = 14.29 by
    Cauchy-Schwarz, so exp() stays within fp32 range (max e^14.3=1.6e6,
    row-sum <= 1.3e10 << fp32 max).  Single-pass softmax.
  - the eps clamp of the reference (||f|| >= 1e-8) is a mathematical
    no-op for these inputs (||f||^2 ~ chi2(768), concentrated at ~768)
    and is skipped.
  - f1 is NOT normalized before the GEMM; inv-norm/TEMP rides in as the
    per-partition `scale` operand of the fused Exp activation.
  - f2 IS normalized pre-GEMM (its inv-norm varies along the free dim).
    Sum-of-squares per f2-row is computed with a ones[128,128] matmul on
    the tensor engine (which also broadcasts the result across all 128
    partitions for free); inv-norm = Exp(-0.5*Ln(x)) so the whole kernel
    uses the single natural_log_exp ACT table set (Rsqrt activation is
    banned for accuracy in this stack).
  - fp32->bf16 casts of the GEMM operands happen inside the SWDGE DMA.
"""

import sys

for _p in ("/opt/trn_rl_repo",):
    if _p not in sys.path:
        sys.path.insert(0, _p)

from contextlib import ExitStack

import numpy as np

import concourse.bass as bass
import concourse.tile as tile
from concourse import mybir

FP32 = mybir.dt.float32
BF16 = mybir.dt.bfloat16
AF = mybir.ActivationFunctionType
ALU = mybir.AluOpType
AX = mybir.AxisListType

N = 8192        # rows of f1/f2
D = 768         # feature dim
NCORES = 8
MC = N // NCORES        # f1 rows per core (1024)
KT = D // 128           # contraction k-chunks (6)
MT = MC // 128          # f1 row tiles per core (8)
PAIR = 1024             # f2 rows processed per outer step
NPAIR = N // PAIR       # 8
TEMP = 0.07
LOG_INV_TEMP = float(-np.log(TEMP))


_WAIT_SPLIT_SKIP = (
    "InstEventSemaphore",
    "InstHalt",
)


def _split_excess_waits(nc: bass.Bass, cap: int = 1) -> None:
    """Hoist per-instruction sync waits beyond `cap` into standalone
    InstEventSemaphore instructions on the same engine.

    The 64-byte TPB instruction encodings carry very few embedded wait
    slots (one for TensorTensor, two for the DMA pseudo-ops, ...) and
    walrus codegen hard-fails on overflow ("Too many sync wait commands").
    Tile's scheduler happily attaches more, so we split them here.
    """
    n = 0
    for bb in nc.main_func.blocks:
        new_list = []
        for inst in bb.instructions:
            si = inst.sync_info
            ow = list(si.on_wait) if si is not None and si.on_wait else []
            if len(ow) > cap and type(inst).__name__ not in _WAIT_SPLIT_SKIP:
                excess, keep = ow[:-cap], ow[-cap:]
                for w in excess:
                    n += 1
                    ev = mybir.InstEventSemaphore(
                        name=f"I-waitsplit-{n}",
                        engine=inst.engine,
                        ins=[],
                        outs=[],
                        sync_info=mybir.SyncInfo(on_wait=[w], on_update=[]),
                    )
                    nc.register_instruction(ev)
                    new_list.append(ev)
                si.on_wait = keep
            new_list.append(inst)
        bb.instructions[:] = new_list


def build_program() -> bass.Bass:
    nc = bass.Bass()
    f1n = nc.declare_dram_parameter("f1n", [MC, D], FP32, isOutput=False)
    f1t = nc.declare_dram_parameter("f1t", [D, MC], FP32, isOutput=False)
    f2t = nc.declare_dram_parameter("f2t", [D, N], FP32, isOutput=False)
    f2dn = nc.declare_dram_parameter("f2dn", [MC, D], FP32, isOutput=False)
    out = nc.declare_dram_parameter("out", [128, MT], FP32, isOutput=True)

    with tile.TileContext(nc, pool_alloc_mode="queue") as tc, ExitStack() as ctx:
        singles = ctx.enter_context(tc.tile_pool(name="singles", bufs=1))

        ones = singles.tile([128, 128], BF16, tag="ones", name="ones")
        nc.any.memset(ones[:], 1.0)
        lbias = singles.tile([128, 1], FP32, tag="lbias", name="lbias")
        nc.any.memset(lbias[:], LOG_INV_TEMP)
        hbias = singles.tile([128, 1], FP32, tag="hbias", name="hbias")
        nc.any.memset(hbias[:], float(np.log(0.5)))

        # resident bf16 operands: f1^T (raw) and normalized f2^T
        n1t = [
            singles.tile([128, MC], BF16, tag=f"n1t{k}", name=f"n1t{k}")
            for k in range(KT)
        ]
        invn1T = singles.tile([128, MT], FP32, tag="invn1T", name="invn1T")
        dvals = singles.tile([128, MT], FP32, tag="dvals", name="dvals")
        spart = singles.tile([128, MT * NPAIR], FP32, tag="spart", name="spart")

        n2p = ctx.enter_context(tc.tile_pool(name="n2p", bufs=1))
        n2st = [
            n2p.tile([128, N], BF16, tag=f"n2st{k}", name=f"n2st{k}")
            for k in range(KT)
        ]

        # ---- hoisted loads: issue every SWDGE stream DMA up front ----
        # f1^T cast to bf16 during DMA (SWDGE cast); needed by the first MMs
        for k in range(KT):
            nc.gpsimd.dma_start(n1t[k][:], f1t[k * 128 : (k + 1) * 128, :])
        # f2^T cast-loads straight into fresh columns of the resident n2st
        f2t_r = f2t[:, :].rearrange("(k p) n -> p k n", p=128)  # [128, KT, N]
        for pair in range(NPAIR):
            c0 = pair * PAIR
            for k in range(KT):
                nc.gpsimd.dma_start(
                    n2st[k][:, c0 : c0 + PAIR], f2t_r[:, k, c0 : c0 + PAIR]
                )

        # ---- P0a: f1-norm prep (early — feeds the first Exp scale).
        # a/b ride the HWDGE (sync) queue so the Pool queue stays dedicated
        # to the f2 stream.  The diagonal (a+b) chain is deferred to the
        # tail (P0b below): its SWDGE accum-DMAs sit behind the 48 stream
        # loads in the Pool FIFO, and its results are only needed at the
        # final subtraction.
        p0s = ctx.enter_context(tc.tile_pool(name="p0s", bufs=1))
        ss1 = p0s.tile([128, MT], FP32, tag="ss1", name="ss1")
        ss2 = p0s.tile([128, MT], FP32, tag="ss2", name="ss2")
        sssum = p0s.tile([128, MT], FP32, tag="sssum", name="sssum")
        draw = p0s.tile([128, MT], FP32, tag="draw", name="draw")
        ab_all = p0s.tile([128, MT, D], FP32, tag="ab_all", name="ab_all")
        with tc.tile_pool(name="pe0", bufs=1) as pe0:
            a_all = pe0.tile([128, MT, D], FP32, tag="a_all", name="a_all")
            b_all = pe0.tile([128, MT, D], FP32, tag="b_all", name="b_all")
            for m in range(MT):
                nc.sync.dma_start(a_all[:, m, :], f1n[m * 128 : (m + 1) * 128, :])
                nc.sync.dma_start(b_all[:, m, :], f2dn[m * 128 : (m + 1) * 128, :])
            for m in range(MT):
                sqa = pe0.tile([128, D], BF16, tag="sqa", name="sqa", bufs=2)
                nc.scalar.activation(
                    sqa[:], a_all[:, m, :], AF.Square, accum_out=ss1[:, m : m + 1]
                )
                sqb = pe0.tile([128, D], BF16, tag="sqb", name="sqb", bufs=2)
                nc.scalar.activation(
                    sqb[:], b_all[:, m, :], AF.Square, accum_out=ss2[:, m : m + 1]
                )
        # invn1T = exp(-0.5*ln(ss1) + ln(1/T)) = 1/(T*||f1_i||)
        t1 = p0s.tile([128, MT], FP32, tag="t1", name="t1")
        nc.scalar.activation(t1[:], ss1[:], AF.Ln)
        nc.scalar.activation(invn1T[:], t1[:], AF.Exp, scale=-0.5, bias=lbias[:])
        # invn2d carries the polarization 0.5: 0.5/||f2d_i||
        t2 = p0s.tile([128, MT], FP32, tag="t2", name="t2")
        nc.scalar.activation(t2[:], ss2[:], AF.Ln)
        invn2d = p0s.tile([128, MT], FP32, tag="invn2d", name="invn2d")
        nc.scalar.activation(invn2d[:], t2[:], AF.Exp, scale=-0.5, bias=hbias[:])

        # ---- pair pipeline: normalize chunk, GEMM, fused exp/row-sum ----
        with tc.tile_pool(name="wk", bufs=2) as wp, tc.tile_pool(
            name="pss", bufs=2, space="PSUM"
        ) as pp, tc.tile_pool(name="psl", bufs=2, space="PSUM") as pl, tc.tile_pool(
            name="expp", bufs=3
        ) as ep:
            for pair in range(NPAIR):
                c0 = pair * PAIR
                sq = wp.tile([128, KT, PAIR], BF16, tag="sq", name="sq")
                for k in range(KT):
                    nc.vector.tensor_mul(
                        sq[:, k, :],
                        n2st[k][:, c0 : c0 + PAIR],
                        n2st[k][:, c0 : c0 + PAIR],
                    )
                # per-f2-row sum of squares, broadcast to all 128 partitions
                ss = pp.tile([128, PAIR], FP32, tag="ss", name="ss")
                for k in range(KT):
                    for h in range(2):
                        nc.tensor.matmul(
                            ss[:, h * 512 : (h + 1) * 512],
                            ones[:],
                            sq[:, k, h * 512 : (h + 1) * 512],
                            start=(k == 0),
                            stop=(k == KT - 1),
                        )
                lntmp = wp.tile([128, PAIR], FP32, tag="lntmp", name="lntmp")
                nc.scalar.activation(lntmp[:], ss[:], AF.Ln)
                invn2 = wp.tile([128, PAIR], BF16, tag="invn2", name="invn2")
                nc.scalar.activation(invn2[:], lntmp[:], AF.Exp, scale=-0.5)
                # normalize the resident chunk in place
                for k in range(KT):
                    nc.vector.tensor_mul(
                        n2st[k][:, c0 : c0 + PAIR],
                        n2st[k][:, c0 : c0 + PAIR],
                        invn2[:],
                    )
                # main GEMM + fused exp/row-sum for every f1 row-tile
                for m in range(MT):
                    pslog = pl.tile([128, PAIR], FP32, tag="pslog", name="pslog")
                    for k in range(KT):
                        for h in range(2):
                            nc.tensor.matmul(
                                pslog[:, h * 512 : (h + 1) * 512],
                                n1t[k][:, m * 128 : (m + 1) * 128],
                                n2st[k][:, c0 + h * 512 : c0 + (h + 1) * 512],
                                start=(k == 0),
                                stop=(k == KT - 1),
                            )
                    eb = ep.tile([128, PAIR], BF16, tag="eb", name="eb")
                    col = m * NPAIR + pair
                    nc.scalar.activation(
                        eb[:],
                        pslog[:],
                        AF.Exp,
                        scale=invn1T[:, m : m + 1],
                        accum_out=spart[:, col : col + 1],
                    )

            # ---- P0b: diagonal chain (tail) ----
            # <a,b> via polarization: a+b formed by accumulating SWDGE DMA
            # (a TensorTensor add would exceed the TT wait-slot budget).
            for m in range(MT):
                nc.gpsimd.dma_start(
                    ab_all[:, m, :], f1n[m * 128 : (m + 1) * 128, :]
                )
                nc.gpsimd.dma_start(
                    ab_all[:, m, :],
                    f2dn[m * 128 : (m + 1) * 128, :],
                    accum_op=ALU.add,
                )
            for m in range(MT):
                sqc = ep.tile([128, D], BF16, tag="sqc", name="sqc", bufs=2)
                nc.scalar.activation(
                    sqc[:], ab_all[:, m, :], AF.Square, accum_out=sssum[:, m : m + 1]
                )
            # draw = sssum - ss1 - ss2 = 2*<f1_i, f2_i>
            t4 = ep.tile([128, MT], FP32, tag="t4", name="t4")
            nc.vector.tensor_sub(t4[:], sssum[:], ss1[:])
            nc.vector.tensor_sub(draw[:], t4[:], ss2[:])
            # dvals = draw * invn1T * invn2d   (logit value on the diagonal)
            t3 = ep.tile([128, MT], FP32, tag="t3", name="t3")
            nc.vector.tensor_mul(t3[:], draw[:], invn1T[:])
            nc.vector.tensor_mul(dvals[:], t3[:], invn2d[:])

            # ---- finalize ----
            S = ep.tile([128, MT], FP32, tag="S", name="S")
            nc.vector.reduce_sum(
                S[:], spart[:].rearrange("p (m q) -> p m q", q=NPAIR), axis=AX.X
            )
            lse = ep.tile([128, MT], FP32, tag="lse", name="lse")
            nc.scalar.activation(lse[:], S[:], AF.Ln)
            res = ep.tile([128, MT], FP32, tag="res", name="res")
            nc.vector.tensor_sub(res[:], lse[:], dvals[:])
            nc.sync.dma_start(out[:, :], res[:])

    _split_excess_waits(nc)
    return nc


def make_in_maps(f1: np.ndarray, f2: np.ndarray) -> list[dict[str, np.ndarray]]:
    f1 = np.ascontiguousarray(np.asarray(f1, dtype=np.float32))
    f2 = np.ascontiguousarray(np.asarray(f2, dtype=np.float32))
    assert f1.shape == (N, D) and f2.shape == (N, D)
    f2t = np.ascontiguousarray(f2.T)  # [D, N], shared by all cores
    in_maps = []
    for c in range(NCORES):
        f1c = np.ascontiguousarray(f1[c * MC : (c + 1) * MC])
        in_maps.append(
            {
                "f1n": f1c,
                "f1t": np.ascontiguousarray(f1c.T),
                "f2t": f2t,
                "f2dn": np.ascontiguousarray(f2[c * MC : (c + 1) * MC]),
            }
        )
    return in_maps


def combine_outputs(outs: list[np.ndarray]) -> np.float32:
    total = 0.0
    for o in outs:
        total += float(np.sum(np.asarray(o, dtype=np.float64)))
    return np.float32(total / float(N))


def run(f1: np.ndarray, f2: np.ndarray, trace: bool = False):
    from concourse.bass_utils import run_bass_kernel_spmd

    nc = build_program()
    in_maps = make_in_maps(f1, f2)
    r = run_bass_kernel_spmd(nc, in_maps, core_ids=list(range(NCORES)), trace=trace)
    outs = [m["out"] for m in r.results]
    return combine_outputs(outs), r


def kernel(f1: np.ndarray, f2: np.ndarray) -> np.ndarray:
    loss, _ = run(f1, f2, trace=False)
    return loss


if __name__ == "__main__":
    f1 = np.random.randn(N, D).astype(np.float32)
    f2 = np.random.randn(N, D).astype(np.float32)
    print(kernel(f1, f2))


# revision 29
# speedup vs baseline: 1.2677x; 1.0648x over previous
"""Contrastive cosine-similarity softmax-CE loss on 8 trn2 NeuronCores.

reference math:
    n1 = f1 / max(||f1||, eps);  n2 = f2 / max(||f2||, eps)
    logits = (n1 @ n2.T) / TEMP                      # [8192, 8192]
    loss = mean_i( logsumexp_j(logits[i, :]) - logits[i, i] )

sharding: f1 rows data-parallel across 8 cores (1024 rows each); f2
replicated (each core streams all of f2 from its HBM copy).  Per-core
output is the vector of per-row (lse - l_ii); host averages.

Device-side algorithm per core (all SPMD-uniform, no collectives):
  - logits are never max-subtracted: |logit| <= 1/0.07 = 14.29 by
    Cauchy-Schwarz, so exp() stays within fp32 range (max e^14.3=1.6e6,
    row-sum <= 1.3e10 << fp32 max).  Single-pass softmax.
  - the eps clamp of the reference (||f|| >= 1e-8) is a mathematical
    no-op for these inputs (||f||^2 ~ chi2(768), concentrated at ~768)
    and is skipped.
  - f1 is NOT normalized before the GEMM; inv-norm/TEMP rides in as the
    per-partition `scale` operand of the fused Exp activation.
  - f2 IS normalized pre-GEMM (its inv-norm varies along the free dim).
    Sum-of-squares per f2-row is computed with a ones[128,128] matmul on
    the tensor engine (which also broadcasts the result across all 128
    partitions for free); inv-norm = Exp(-0.5*Ln(x)) so the whole kernel
    uses the single natural_log_exp ACT table set (Rsqrt activation is
    banned for accuracy in this stack).
  - fp32->bf16 casts of the GEMM operands happen inside the SWDGE DMA.
"""

import sys

for _p in ("/opt/trn_rl_repo",):
    if _p not in sys.path:
        sys.path.insert(0, _p)

from contextlib import ExitStack

import numpy as np

import concourse.bass as bass
import concourse.tile as tile
from concourse import mybir

FP32 = mybir.dt.float32
BF16 = mybir.dt.bfloat16
AF = mybir.ActivationFunctionType
ALU = mybir.AluOpType
AX = mybir.AxisListType

N = 8192        # rows of f1/f2
D = 768         # feature dim
NCORES = 8
MC = N // NCORES        # f1 rows per core (1024)
KT = D // 128           # contraction k-chunks (6)
MT = MC // 128          # f1 row tiles per core (8)
PAIR = 1024             # f2 rows processed per outer step
NPAIR = N // PAIR       # 8
TEMP = 0.07
LOG_INV_TEMP = float(-np.log(TEMP))


_WAIT_SPLIT_SKIP = (
    "InstEventSemaphore",
    "InstHalt",
)


def _split_excess_waits(nc: bass.Bass, cap: int = 1) -> None:
    """Hoist per-instruction sync waits beyond `cap` into standalone
    InstEventSemaphore instructions on the same engine.

    The 64-byte TPB instruction encodings carry very few embedded wait
    slots (one for TensorTensor, two for the DMA pseudo-ops, ...) and
    walrus codegen hard-fails on overflow ("Too many sync wait commands").
    Tile's scheduler happily attaches more, so we split them here.
    """
    n = 0
    for bb in nc.main_func.blocks:
        new_list = []
        for inst in bb.instructions:
            si = inst.sync_info
            ow = list(si.on_wait) if si is not None and si.on_wait else []
            if len(ow) > cap and type(inst).__name__ not in _WAIT_SPLIT_SKIP:
                excess, keep = ow[:-cap], ow[-cap:]
                for w in excess:
                    n += 1
                    ev = mybir.InstEventSemaphore(
                        name=f"I-waitsplit-{n}",
                        engine=inst.engine,
                        ins=[],
                        outs=[],
                        sync_info=mybir.SyncInfo(on_wait=[w], on_update=[]),
                    )
                    nc.register_instruction(ev)
                    new_list.append(ev)
                si.on_wait = keep
            new_list.append(inst)
        bb.instructions[:] = new_list


def build_program() -> bass.Bass:
    nc = bass.Bass()
    f1n = nc.declare_dram_parameter("f1n", [MC, D], FP32, isOutput=False)
    f1t = nc.declare_dram_parameter("f1t", [D, MC], FP32, isOutput=False)
    f2t = nc.declare_dram_parameter("f2t", [D, N], FP32, isOutput=False)
    f2dn = nc.declare_dram_parameter("f2dn", [MC, D], FP32, isOutput=False)
    out = nc.declare_dram_parameter("out", [128, MT], FP32, isOutput=True)

    with tile.TileContext(nc, pool_alloc_mode="queue") as tc, ExitStack() as ctx:
        singles = ctx.enter_context(tc.tile_pool(name="singles", bufs=1))

        ones = singles.tile([128, 128], BF16, tag="ones", name="ones")
        nc.any.memset(ones[:], 1.0)
        lbias = singles.tile([128, 1], FP32, tag="lbias", name="lbias")
        nc.any.memset(lbias[:], LOG_INV_TEMP)
        hbias = singles.tile([128, 1], FP32, tag="hbias", name="hbias")
        nc.any.memset(hbias[:], float(np.log(0.5)))

        # resident bf16 operands: f1^T (raw) and normalized f2^T
        n1t = [
            singles.tile([128, MC], BF16, tag=f"n1t{k}", name=f"n1t{k}")
            for k in range(KT)
        ]
        invn1T = singles.tile([128, MT], FP32, tag="invn1T", name="invn1T")
        dvals = singles.tile([128, MT], FP32, tag="dvals", name="dvals")
        spart = singles.tile([128, MT * NPAIR], FP32, tag="spart", name="spart")

        n2p = ctx.enter_context(tc.tile_pool(name="n2p", bufs=1))
        n2st = [
            n2p.tile([128, N], BF16, tag=f"n2st{k}", name=f"n2st{k}")
            for k in range(KT)
        ]

        # ---- hoisted loads, in consumption-priority order: the SDMA pool
        # round-robins across queued work at packet granularity, so early
        # bytes delay pair-0 readiness 1:1.  pair-0 chunk first, then f1^T
        # (needed by the first main matmul), then the remaining stream.
        f2t_r = f2t[:, :].rearrange("(k p) n -> p k n", p=128)  # [128, KT, N]
        for k in range(KT):
            nc.gpsimd.dma_start(n2st[k][:, 0:PAIR], f2t_r[:, k, 0:PAIR])
        for k in range(KT):
            nc.gpsimd.dma_start(n1t[k][:], f1t[k * 128 : (k + 1) * 128, :])
        for pair in range(1, NPAIR):
            c0 = pair * PAIR
            for k in range(KT):
                nc.gpsimd.dma_start(
                    n2st[k][:, c0 : c0 + PAIR], f2t_r[:, k, c0 : c0 + PAIR]
                )

        # ---- P0a: f1-norm prep (early — feeds the first Exp scale).
        # a/b ride the HWDGE (sync) queue so the Pool queue stays dedicated
        # to the f2 stream.  The diagonal (a+b) chain is deferred to the
        # tail (P0b below): its SWDGE accum-DMAs sit behind the 48 stream
        # loads in the Pool FIFO, and its results are only needed at the
        # final subtraction.
        p0s = ctx.enter_context(tc.tile_pool(name="p0s", bufs=1))
        ss1 = p0s.tile([128, MT], FP32, tag="ss1", name="ss1")
        ss2 = p0s.tile([128, MT], FP32, tag="ss2", name="ss2")
        sssum = p0s.tile([128, MT], FP32, tag="sssum", name="sssum")
        draw = p0s.tile([128, MT], FP32, tag="draw", name="draw")
        ab_all = p0s.tile([128, MT, D], FP32, tag="ab_all", name="ab_all")
        with tc.tile_pool(name="pe0", bufs=1) as pe0:
            a_all = pe0.tile([128, MT, D], FP32, tag="a_all", name="a_all")
            for m in range(MT):
                nc.sync.dma_start(a_all[:, m, :], f1n[m * 128 : (m + 1) * 128, :])
            for m in range(MT):
                sqa = pe0.tile([128, D], BF16, tag="sqa", name="sqa", bufs=2)
                nc.scalar.activation(
                    sqa[:], a_all[:, m, :], AF.Square, accum_out=ss1[:, m : m + 1]
                )
        # invn1T = exp(-0.5*ln(ss1) + ln(1/T)) = 1/(T*||f1_i||)
        t1 = p0s.tile([128, MT], FP32, tag="t1", name="t1")
        nc.scalar.activation(t1[:], ss1[:], AF.Ln)
        nc.scalar.activation(invn1T[:], t1[:], AF.Exp, scale=-0.5, bias=lbias[:])

        # ---- pair pipeline: normalize chunk, GEMM, fused exp/row-sum ----
        with tc.tile_pool(name="wk", bufs=2) as wp, tc.tile_pool(
            name="pss", bufs=2, space="PSUM"
        ) as pp, tc.tile_pool(name="psl", bufs=2, space="PSUM") as pl, tc.tile_pool(
            name="expp", bufs=3
        ) as ep:
            def do_pair(pair):
                c0 = pair * PAIR
                sq = wp.tile([128, KT, PAIR], BF16, tag="sq", name="sq")
                for k in range(KT):
                    nc.vector.tensor_mul(
                        sq[:, k, :],
                        n2st[k][:, c0 : c0 + PAIR],
                        n2st[k][:, c0 : c0 + PAIR],
                    )
                # per-f2-row sum of squares, broadcast to all 128 partitions
                ss = pp.tile([128, PAIR], FP32, tag="ss", name="ss")
                for k in range(KT):
                    for h in range(2):
                        nc.tensor.matmul(
                            ss[:, h * 512 : (h + 1) * 512],
                            ones[:],
                            sq[:, k, h * 512 : (h + 1) * 512],
                            start=(k == 0),
                            stop=(k == KT - 1),
                        )
                lntmp = wp.tile([128, PAIR], FP32, tag="lntmp", name="lntmp")
                nc.scalar.activation(lntmp[:], ss[:], AF.Ln)
                invn2 = wp.tile([128, PAIR], BF16, tag="invn2", name="invn2")
                nc.scalar.activation(invn2[:], lntmp[:], AF.Exp, scale=-0.5)
                # normalize the resident chunk in place
                for k in range(KT):
                    nc.vector.tensor_mul(
                        n2st[k][:, c0 : c0 + PAIR],
                        n2st[k][:, c0 : c0 + PAIR],
                        invn2[:],
                    )
                # main GEMM + fused exp/row-sum for every f1 row-tile
                for m in range(MT):
                    pslog = pl.tile([128, PAIR], FP32, tag="pslog", name="pslog")
                    for k in range(KT):
                        for h in range(2):
                            nc.tensor.matmul(
                                pslog[:, h * 512 : (h + 1) * 512],
                                n1t[k][:, m * 128 : (m + 1) * 128],
                                n2st[k][:, c0 + h * 512 : c0 + (h + 1) * 512],
                                start=(k == 0),
                                stop=(k == KT - 1),
                            )
                    eb = ep.tile([128, PAIR], BF16, tag="eb", name="eb")
                    col = m * NPAIR + pair
                    nc.scalar.activation(
                        eb[:],
                        pslog[:],
                        AF.Exp,
                        scale=invn1T[:, m : m + 1],
                        accum_out=spart[:, col : col + 1],
                    )

            for pair in range(NPAIR - 1):
                do_pair(pair)

            # ---- P0b: diagonal chain (emitted before the last pair so its
            # ACT/DVE work hides under the final matmul wave) ----
            # <a,b> via polarization: a+b formed by accumulating SWDGE DMA
            # (a TensorTensor add would exceed the TT wait-slot budget).
            # ab_all pulls double duty: first holds b (for ss2), then the
            # SWDGE accumulate adds a on top, giving a+b (for sssum).
            for m in range(MT):
                nc.gpsimd.dma_start(
                    ab_all[:, m, :], f2dn[m * 128 : (m + 1) * 128, :]
                )
            for m in range(MT):
                sqb = ep.tile([128, D], BF16, tag="sqb", name="sqb", bufs=2)
                nc.scalar.activation(
                    sqb[:], ab_all[:, m, :], AF.Square, accum_out=ss2[:, m : m + 1]
                )
            for m in range(MT):
                nc.gpsimd.dma_start(
                    ab_all[:, m, :],
                    f1n[m * 128 : (m + 1) * 128, :],
                    accum_op=ALU.add,
                )
            for m in range(MT):
                sqc = ep.tile([128, D], BF16, tag="sqc", name="sqc", bufs=2)
                nc.scalar.activation(
                    sqc[:], ab_all[:, m, :], AF.Square, accum_out=sssum[:, m : m + 1]
                )
            t2 = p0s.tile([128, MT], FP32, tag="t2", name="t2")
            nc.scalar.activation(t2[:], ss2[:], AF.Ln)
            invn2d = p0s.tile([128, MT], FP32, tag="invn2d", name="invn2d")
            nc.scalar.activation(invn2d[:], t2[:], AF.Exp, scale=-0.5, bias=hbias[:])
            # draw = sssum - ss1 - ss2 = 2*<f1_i, f2_i>
            t4 = ep.tile([128, MT], FP32, tag="t4", name="t4", bufs=1)
            nc.vector.tensor_sub(t4[:], sssum[:], ss1[:])
            nc.vector.tensor_sub(draw[:], t4[:], ss2[:])
            # dvals = draw * invn1T * invn2d   (logit value on the diagonal)
            t3 = ep.tile([128, MT], FP32, tag="t3", name="t3", bufs=1)
            nc.vector.tensor_mul(t3[:], draw[:], invn1T[:])
            nc.vector.tensor_mul(dvals[:], t3[:], invn2d[:])

            do_pair(NPAIR - 1)

            # ---- finalize ----
            S = ep.tile([128, MT], FP32, tag="S", name="S", bufs=1)
            nc.vector.reduce_sum(
                S[:], spart[:].rearrange("p (m q) -> p m q", q=NPAIR), axis=AX.X
            )
            lse = ep.tile([128, MT], FP32, tag="lse", name="lse", bufs=1)
            nc.scalar.activation(lse[:], S[:], AF.Ln)
            res = ep.tile([128, MT], FP32, tag="res", name="res", bufs=1)
            nc.vector.tensor_sub(res[:], lse[:], dvals[:])
            nc.sync.dma_start(out[:, :], res[:])

    _split_excess_waits(nc)
    return nc


def make_in_maps(f1: np.ndarray, f2: np.ndarray) -> list[dict[str, np.ndarray]]:
    f1 = np.ascontiguousarray(np.asarray(f1, dtype=np.float32))
    f2 = np.ascontiguousarray(np.asarray(f2, dtype=np.float32))
    assert f1.shape == (N, D) and f2.shape == (N, D)
    f2t = np.ascontiguousarray(f2.T)  # [D, N], shared by all cores
    in_maps = []
    for c in range(NCORES):
        f1c = np.ascontiguousarray(f1[c * MC : (c + 1) * MC])
        in_maps.append(
            {
                "f1n": f1c,
                "f1t": np.ascontiguousarray(f1c.T),
                "f2t": f2t,
                "f2dn": np.ascontiguousarray(f2[c * MC : (c + 1) * MC]),
            }
        )
    return in_maps


def combine_outputs(outs: list[np.ndarray]) -> np.float32:
    total = 0.0
    for o in outs:
        total += float(np.sum(np.asarray(o, dtype=np.float64)))
    return np.float32(total / float(N))


def run(f1: np.ndarray, f2: np.ndarray, trace: bool = False):
    from concourse.bass_utils import run_bass_kernel_spmd

    nc = build_program()
    in_maps = make_in_maps(f1, f2)
    r = run_bass_kernel_spmd(nc, in_maps, core_ids=list(range(NCORES)), trace=trace)
    outs = [m["out"] for m in r.results]
    return combine_outputs(outs), r


def kernel(f1: np.ndarray, f2: np.ndarray) -> np.ndarray:
    loss, _ = run(f1, f2, trace=False)
    return loss


if __name__ == "__main__":
    f1 = np.random.randn(N, D).astype(np.float32)
    f2 = np.random.randn(N, D).astype(np.float32)
    print(kernel(f1, f2))


# revision 35
# speedup vs baseline: 1.3922x; 1.0982x over previous
"""Contrastive cosine-similarity softmax-CE loss on 8 trn2 NeuronCores.

reference math:
    n1 = f1 / max(||f1||, eps);  n2 = f2 / max(||f2||, eps)
    logits = (n1 @ n2.T) / TEMP                      # [8192, 8192]
    loss = mean_i( logsumexp_j(logits[i, :]) - logits[i, i] )

sharding: f1 rows data-parallel across 8 cores (1024 rows each); f2
replicated (each core streams all of f2 from its HBM copy).  Per-core
output is the vector of per-row (lse - l_ii); host averages.

Device-side algorithm per core (all SPMD-uniform, no collectives):
  - logits are never max-subtracted: |logit| <= 1/0.07 = 14.29 by
    Cauchy-Schwarz, so exp() stays within fp32 range (max e^14.3=1.6e6,
    row-sum <= 1.3e10 << fp32 max).  Single-pass softmax.
  - the eps clamp of the reference (||f|| >= 1e-8) is a mathematical
    no-op for these inputs (||f||^2 ~ chi2(768), concentrated at ~768)
    and is skipped.
  - f1 is NOT normalized before the GEMM; inv-norm/TEMP rides in as the
    per-partition `scale` operand of the fused Exp activation.
  - f2 IS normalized pre-GEMM (its inv-norm varies along the free dim).
    Sum-of-squares per f2-row is computed with a ones[128,128] matmul on
    the tensor engine (which also broadcasts the result across all 128
    partitions for free); inv-norm = Exp(-0.5*Ln(x)) so the whole kernel
    uses the single natural_log_exp ACT table set (Rsqrt activation is
    banned for accuracy in this stack).
  - fp32->bf16 casts of the GEMM operands happen inside the SWDGE DMA.
"""

import sys

for _p in ("/opt/trn_rl_repo",):
    if _p not in sys.path:
        sys.path.insert(0, _p)

from contextlib import ExitStack

import numpy as np

import concourse.bass as bass
import concourse.tile as tile
from concourse import mybir

FP32 = mybir.dt.float32
BF16 = mybir.dt.bfloat16
AF = mybir.ActivationFunctionType
ALU = mybir.AluOpType
AX = mybir.AxisListType

N = 8192        # rows of f1/f2
D = 768         # feature dim
NCORES = 8
MC = N // NCORES        # f1 rows per core (1024)
KT = D // 128           # contraction k-chunks (6)
MT = MC // 128          # f1 row tiles per core (8)
PAIR = 1024             # f2 rows processed per outer step
NPAIR = N // PAIR       # 8
TEMP = 0.07
LOG_INV_TEMP = float(-np.log(TEMP))


_WAIT_SPLIT_SKIP = (
    "InstEventSemaphore",
    "InstHalt",
)


def _split_excess_waits(nc: bass.Bass, cap: int = 1) -> None:
    """Hoist per-instruction sync waits beyond `cap` into standalone
    InstEventSemaphore instructions on the same engine.

    The 64-byte TPB instruction encodings carry very few embedded wait
    slots (one for TensorTensor, two for the DMA pseudo-ops, ...) and
    walrus codegen hard-fails on overflow ("Too many sync wait commands").
    Tile's scheduler happily attaches more, so we split them here.
    """
    n = 0
    for bb in nc.main_func.blocks:
        new_list = []
        for inst in bb.instructions:
            si = inst.sync_info
            ow = list(si.on_wait) if si is not None and si.on_wait else []
            if len(ow) > cap and type(inst).__name__ not in _WAIT_SPLIT_SKIP:
                excess, keep = ow[:-cap], ow[-cap:]
                for w in excess:
                    n += 1
                    ev = mybir.InstEventSemaphore(
                        name=f"I-waitsplit-{n}",
                        engine=inst.engine,
                        ins=[],
                        outs=[],
                        sync_info=mybir.SyncInfo(on_wait=[w], on_update=[]),
                    )
                    nc.register_instruction(ev)
                    new_list.append(ev)
                si.on_wait = keep
            new_list.append(inst)
        bb.instructions[:] = new_list


def build_program() -> bass.Bass:
    nc = bass.Bass()
    f1n = nc.declare_dram_parameter("f1n", [MC, D], FP32, isOutput=False)
    f1t = nc.declare_dram_parameter("f1t", [D, MC], FP32, isOutput=False)
    f2tp = nc.declare_dram_parameter(
        "f2tp", [NPAIR, 128, KT, PAIR], FP32, isOutput=False
    )
    f2dn = nc.declare_dram_parameter("f2dn", [MC, D], FP32, isOutput=False)
    out = nc.declare_dram_parameter("out", [128, MT], FP32, isOutput=True)

    with tile.TileContext(nc, pool_alloc_mode="queue") as tc, ExitStack() as ctx:
        singles = ctx.enter_context(tc.tile_pool(name="singles", bufs=1))

        ones = singles.tile([128, 128], BF16, tag="ones", name="ones")
        nc.any.memset(ones[:], 1.0)
        lbias = singles.tile([128, 1], FP32, tag="lbias", name="lbias")
        nc.any.memset(lbias[:], LOG_INV_TEMP)
        hbias = singles.tile([128, 1], FP32, tag="hbias", name="hbias")
        nc.any.memset(hbias[:], float(np.log(0.5)))

        # resident bf16 operands: f1^T (raw) and normalized f2^T
        n1t = [
            singles.tile([128, MC], BF16, tag=f"n1t{k}", name=f"n1t{k}")
            for k in range(KT)
        ]
        invn1T = singles.tile([128, MT], FP32, tag="invn1T", name="invn1T")
        dvals = singles.tile([128, MT], FP32, tag="dvals", name="dvals")
        spart = singles.tile([128, MT * NPAIR], FP32, tag="spart", name="spart")

        n2p = ctx.enter_context(tc.tile_pool(name="n2p", bufs=1))
        n2t = n2p.tile([128, KT, N], BF16, tag="n2t", name="n2t")

        pe0 = ctx.enter_context(tc.tile_pool(name="pe0", bufs=1))
        a_all = pe0.tile([128, MT, D], BF16, tag="a_all", name="a_all")

        # ---- hoisted loads, in consumption-priority order: the SDMA pool
        # round-robins across queued work at packet granularity, so early
        # bytes delay pair-0 readiness 1:1.  pair-0 chunk first, then f1^T
        # (needed by the first main matmul), then the remaining stream.
        # f2tp is packed [pair][p][k][n] on the host so each pair is one
        # 3 MB DMA whose per-partition source reads are 24 KB contiguous.
        nc.gpsimd.dma_start(n2t[:, :, 0:PAIR], f2tp[0])
        for k in range(KT):
            nc.gpsimd.dma_start(n1t[k][:], f1t[k * 128 : (k + 1) * 128, :])
        for m in range(MT):
            nc.gpsimd.dma_start(a_all[:, m, :], f1n[m * 128 : (m + 1) * 128, :])
        for pair in range(1, NPAIR):
            c0 = pair * PAIR
            nc.gpsimd.dma_start(n2t[:, :, c0 : c0 + PAIR], f2tp[pair])

        # ---- P0a: f1-norm prep (early — feeds the first Exp scale).
        # a/b ride the HWDGE (sync) queue so the Pool queue stays dedicated
        # to the f2 stream.  The diagonal (a+b) chain is deferred to the
        # tail (P0b below): its SWDGE accum-DMAs sit behind the 48 stream
        # loads in the Pool FIFO, and its results are only needed at the
        # final subtraction.
        p0s = ctx.enter_context(tc.tile_pool(name="p0s", bufs=1))
        ss1 = p0s.tile([128, MT], FP32, tag="ss1", name="ss1")
        ss2 = p0s.tile([128, MT], FP32, tag="ss2", name="ss2")
        sssum = p0s.tile([128, MT], FP32, tag="sssum", name="sssum")
        draw = p0s.tile([128, MT], FP32, tag="draw", name="draw")

        def p0a_act():
            # emitted after pair-0's normalize chain so these squares don't
            # clog the ACT FIFO ahead of it; invn1T is only consumed by the
            # (later) first Exp.
            for m in range(MT):
                sqa = pe0.tile([128, D], BF16, tag="sqa", name="sqa", bufs=2)
                nc.scalar.activation(
                    sqa[:], a_all[:, m, :], AF.Square, accum_out=ss1[:, m : m + 1]
                )
            # invn1T = exp(-0.5*ln(ss1) + ln(1/T)) = 1/(T*||f1_i||)
            t1 = p0s.tile([128, MT], FP32, tag="t1", name="t1")
            nc.scalar.activation(t1[:], ss1[:], AF.Ln)
            nc.scalar.activation(invn1T[:], t1[:], AF.Exp, scale=-0.5, bias=lbias[:])

        # ---- pair pipeline: normalize chunk, GEMM, fused exp/row-sum ----
        with tc.tile_pool(name="wk", bufs=2) as wp, tc.tile_pool(
            name="pss", bufs=2, space="PSUM"
        ) as pp, tc.tile_pool(name="psl", bufs=2, space="PSUM") as pl, tc.tile_pool(
            name="expp", bufs=3
        ) as ep:
            def do_pair(pair, mid=None):
                c0 = pair * PAIR
                sq = wp.tile([128, KT, PAIR], BF16, tag="sq", name="sq")
                for k in range(KT):
                    nc.vector.tensor_mul(
                        sq[:, k, :],
                        n2t[:, k, c0 : c0 + PAIR],
                        n2t[:, k, c0 : c0 + PAIR],
                    )
                # per-f2-row sum of squares, broadcast to all 128 partitions
                ss = pp.tile([128, PAIR], FP32, tag="ss", name="ss")
                for k in range(KT):
                    for h in range(2):
                        nc.tensor.matmul(
                            ss[:, h * 512 : (h + 1) * 512],
                            ones[:],
                            sq[:, k, h * 512 : (h + 1) * 512],
                            start=(k == 0),
                            stop=(k == KT - 1),
                        )
                lntmp = wp.tile([128, PAIR], FP32, tag="lntmp", name="lntmp", bufs=1)
                nc.scalar.activation(lntmp[:], ss[:], AF.Ln)
                invn2 = wp.tile([128, PAIR], BF16, tag="invn2", name="invn2")
                nc.scalar.activation(invn2[:], lntmp[:], AF.Exp, scale=-0.5)
                # normalize the resident chunk in place
                for k in range(KT):
                    nc.vector.tensor_mul(
                        n2t[:, k, c0 : c0 + PAIR],
                        n2t[:, k, c0 : c0 + PAIR],
                        invn2[:],
                    )
                if mid is not None:
                    mid()
                # main GEMM + fused exp/row-sum for every f1 row-tile
                for m in range(MT):
                    pslog = pl.tile([128, PAIR], FP32, tag="pslog", name="pslog")
                    for k in range(KT):
                        for h in range(2):
                            nc.tensor.matmul(
                                pslog[:, h * 512 : (h + 1) * 512],
                                n1t[k][:, m * 128 : (m + 1) * 128],
                                n2t[:, k, c0 + h * 512 : c0 + (h + 1) * 512],
                                start=(k == 0),
                                stop=(k == KT - 1),
                            )
                    eb = ep.tile([128, PAIR], BF16, tag="eb", name="eb")
                    col = m * NPAIR + pair
                    nc.scalar.activation(
                        eb[:],
                        pslog[:],
                        AF.Exp,
                        scale=invn1T[:, m : m + 1],
                        accum_out=spart[:, col : col + 1],
                    )

            do_pair(0, mid=p0a_act)
            for pair in range(1, NPAIR - 1):
                do_pair(pair)

            # ---- P0b: diagonal chain (emitted before the last pair so its
            # ACT/DVE work hides under the final matmul wave) ----
            # <a,b> via polarization: a+b formed by accumulating SWDGE DMA
            # (a TensorTensor add would exceed the TT wait-slot budget).
            # ab_all pulls double duty: first holds b (for ss2), then the
            # SWDGE accumulate adds a on top, giving a+b (for sssum).
            pab_ctx = ExitStack()
            pab = pab_ctx.enter_context(tc.tile_pool(name="pab", bufs=1))
            ab_all = pab.tile([128, MT, D], FP32, tag="ab_all", name="ab_all")
            for m in range(MT):
                nc.gpsimd.dma_start(
                    ab_all[:, m, :], f2dn[m * 128 : (m + 1) * 128, :]
                )
            for m in range(MT):
                sqb = ep.tile([128, D], BF16, tag="sqb", name="sqb", bufs=2)
                nc.scalar.activation(
                    sqb[:], ab_all[:, m, :], AF.Square, accum_out=ss2[:, m : m + 1]
                )
            for m in range(MT):
                nc.gpsimd.dma_start(
                    ab_all[:, m, :],
                    f1n[m * 128 : (m + 1) * 128, :],
                    accum_op=ALU.add,
                )
            for m in range(MT):
                sqc = ep.tile([128, D], BF16, tag="sqc", name="sqc", bufs=2)
                nc.scalar.activation(
                    sqc[:], ab_all[:, m, :], AF.Square, accum_out=sssum[:, m : m + 1]
                )
            t2 = p0s.tile([128, MT], FP32, tag="t2", name="t2")
            nc.scalar.activation(t2[:], ss2[:], AF.Ln)
            invn2d = p0s.tile([128, MT], FP32, tag="invn2d", name="invn2d")
            nc.scalar.activation(invn2d[:], t2[:], AF.Exp, scale=-0.5, bias=hbias[:])
            # draw = sssum - ss1 - ss2 = 2*<f1_i, f2_i>
            t4 = ep.tile([128, MT], FP32, tag="t4", name="t4", bufs=1)
            nc.vector.tensor_sub(t4[:], sssum[:], ss1[:])
            nc.vector.tensor_sub(draw[:], t4[:], ss2[:])
            # dvals = draw * invn1T * invn2d   (logit value on the diagonal)
            t3 = ep.tile([128, MT], FP32, tag="t3", name="t3", bufs=1)
            nc.vector.tensor_mul(t3[:], draw[:], invn1T[:])
            nc.vector.tensor_mul(dvals[:], t3[:], invn2d[:])

            do_pair(NPAIR - 1)

            # ---- finalize ----
            S = ep.tile([128, MT], FP32, tag="S", name="S", bufs=1)
            nc.vector.reduce_sum(
                S[:], spart[:].rearrange("p (m q) -> p m q", q=NPAIR), axis=AX.X
            )
            lse = ep.tile([128, MT], FP32, tag="lse", name="lse", bufs=1)
            nc.scalar.activation(lse[:], S[:], AF.Ln)
            res = ep.tile([128, MT], FP32, tag="res", name="res", bufs=1)
            nc.vector.tensor_sub(res[:], lse[:], dvals[:])
            nc.sync.dma_start(out[:, :], res[:])
            pab_ctx.close()

    _split_excess_waits(nc)
    return nc


def make_in_maps(f1: np.ndarray, f2: np.ndarray) -> list[dict[str, np.ndarray]]:
    f1 = np.ascontiguousarray(np.asarray(f1, dtype=np.float32))
    f2 = np.ascontiguousarray(np.asarray(f2, dtype=np.float32))
    assert f1.shape == (N, D) and f2.shape == (N, D)
    f2t = f2.T  # [D, N]
    # pack pair-major, partition-major: f2tp[q, p, k, n] = f2t[k*128+p, q*1024+n]
    f2tp = np.ascontiguousarray(
        f2t.reshape(KT, 128, NPAIR, PAIR).transpose(2, 1, 0, 3)
    )
    in_maps = []
    for c in range(NCORES):
        f1c = np.ascontiguousarray(f1[c * MC : (c + 1) * MC])
        in_maps.append(
            {
                "f1n": f1c,
                "f1t": np.ascontiguousarray(f1c.T),
                "f2tp": f2tp,
                "f2dn": np.ascontiguousarray(f2[c * MC : (c + 1) * MC]),
            }
        )
    return in_maps


def combine_outputs(outs: list[np.ndarray]) -> np.float32:
    total = 0.0
    for o in outs:
        total += float(np.sum(np.asarray(o, dtype=np.float64)))
    return np.float32(total / float(N))


def run(f1: np.ndarray, f2: np.ndarray, trace: bool = False):
    from concourse.bass_utils import run_bass_kernel_spmd

    nc = build_program()
    in_maps = make_in_maps(f1, f2)
    r = run_bass_kernel_spmd(nc, in_maps, core_ids=list(range(NCORES)), trace=trace)
    outs = [m["out"] for m in r.results]
    return combine_outputs(outs), r


def kernel(f1: np.ndarray, f2: np.ndarray) -> np.ndarray:
    loss, _ = run(f1, f2, trace=False)
    return loss


if __name__ == "__main__":
    f1 = np.random.randn(N, D).astype(np.float32)
    f2 = np.random.randn(N, D).astype(np.float32)
    print(kernel(f1, f2))


# revision 37
# speedup vs baseline: 1.6661x; 1.1967x over previous
"""Contrastive cosine-similarity softmax-CE loss on 8 trn2 NeuronCores.

reference math:
    n1 = f1 / max(||f1||, eps);  n2 = f2 / max(||f2||, eps)
    logits = (n1 @ n2.T) / TEMP                      # [8192, 8192]
    loss = mean_i( logsumexp_j(logits[i, :]) - logits[i, i] )

sharding: f1 rows data-parallel across 8 cores (1024 rows each); f2
replicated (each core streams all of f2 from its HBM copy).  Per-core
output is the vector of per-row (lse - l_ii); host averages.

Device-side algorithm per core (all SPMD-uniform, no collectives):
  - logits are never max-subtracted: |logit| <= 1/0.07 = 14.29 by
    Cauchy-Schwarz, so exp() stays within fp32 range (max e^14.3=1.6e6,
    row-sum <= 1.3e10 << fp32 max).  Single-pass softmax.
  - the eps clamp of the reference (||f|| >= 1e-8) is a mathematical
    no-op for these inputs (||f||^2 ~ chi2(768), concentrated at ~768)
    and is skipped.
  - f1 is NOT normalized before the GEMM; inv-norm/TEMP rides in as the
    per-partition `scale` operand of the fused Exp activation.
  - f2 IS normalized pre-GEMM (its inv-norm varies along the free dim).
    Sum-of-squares per f2-row is computed with a ones[128,128] matmul on
    the tensor engine (which also broadcasts the result across all 128
    partitions for free); inv-norm = Exp(-0.5*Ln(x)) so the whole kernel
    uses the single natural_log_exp ACT table set (Rsqrt activation is
    banned for accuracy in this stack).
  - fp32->bf16 casts of the GEMM operands happen inside the SWDGE DMA.
"""

import sys

for _p in ("/opt/trn_rl_repo",):
    if _p not in sys.path:
        sys.path.insert(0, _p)

from contextlib import ExitStack

import numpy as np

import concourse.bass as bass
import concourse.tile as tile
from concourse import mybir

FP32 = mybir.dt.float32
BF16 = mybir.dt.bfloat16
AF = mybir.ActivationFunctionType
ALU = mybir.AluOpType
AX = mybir.AxisListType

N = 8192        # rows of f1/f2
D = 768         # feature dim
NCORES = 8
MC = N // NCORES        # f1 rows per core (1024)
KT = D // 128           # contraction k-chunks (6)
MT = MC // 128          # f1 row tiles per core (8)
PAIR = 1024             # f2 rows processed per outer step
NPAIR = N // PAIR       # 8
TEMP = 0.07
LOG_INV_TEMP = float(-np.log(TEMP))
FP8 = mybir.dt.float8e4
SC = 32.0  # power-of-2 prescale for fp8 f2 operand


_WAIT_SPLIT_SKIP = (
    "InstEventSemaphore",
    "InstHalt",
)


def _split_excess_waits(nc: bass.Bass, cap: int = 1) -> None:
    """Hoist per-instruction sync waits beyond `cap` into standalone
    InstEventSemaphore instructions on the same engine.

    The 64-byte TPB instruction encodings carry very few embedded wait
    slots (one for TensorTensor, two for the DMA pseudo-ops, ...) and
    walrus codegen hard-fails on overflow ("Too many sync wait commands").
    Tile's scheduler happily attaches more, so we split them here.
    """
    n = 0
    for bb in nc.main_func.blocks:
        new_list = []
        for inst in bb.instructions:
            si = inst.sync_info
            ow = list(si.on_wait) if si is not None and si.on_wait else []
            if len(ow) > cap and type(inst).__name__ not in _WAIT_SPLIT_SKIP:
                excess, keep = ow[:-cap], ow[-cap:]
                for w in excess:
                    n += 1
                    ev = mybir.InstEventSemaphore(
                        name=f"I-waitsplit-{n}",
                        engine=inst.engine,
                        ins=[],
                        outs=[],
                        sync_info=mybir.SyncInfo(on_wait=[w], on_update=[]),
                    )
                    nc.register_instruction(ev)
                    new_list.append(ev)
                si.on_wait = keep
            new_list.append(inst)
        bb.instructions[:] = new_list


def build_program() -> bass.Bass:
    nc = bass.Bass()
    f1n = nc.declare_dram_parameter("f1n", [MC, D], FP32, isOutput=False)
    f1t = nc.declare_dram_parameter("f1t", [D, MC], FP32, isOutput=False)
    f2tp = nc.declare_dram_parameter(
        "f2tp", [NPAIR, 128, KT, PAIR], FP32, isOutput=False
    )
    f2dn = nc.declare_dram_parameter("f2dn", [MC, D], FP32, isOutput=False)
    out = nc.declare_dram_parameter("out", [128, MT], FP32, isOutput=True)

    with tile.TileContext(nc, pool_alloc_mode="queue") as tc, ExitStack() as ctx:
        singles = ctx.enter_context(tc.tile_pool(name="singles", bufs=1))

        ones = singles.tile([128, 128], BF16, tag="ones", name="ones")
        nc.any.memset(ones[:], 1.0)
        # exp scale bias: ln(1/TEMP) - ln(SC)  (SC un-scales the fp8 prescale)
        lbias = singles.tile([128, 1], FP32, tag="lbias", name="lbias")
        nc.any.memset(lbias[:], LOG_INV_TEMP - float(np.log(SC)))
        # invn2d bias: ln(0.5 * SC) — the 0.5 is the polarization factor,
        # the SC cancels the 1/SC folded into invn1T (dvals uses both)
        hbias = singles.tile([128, 1], FP32, tag="hbias", name="hbias")
        nc.any.memset(hbias[:], float(np.log(0.5 * SC)))
        # invn2 bias: +ln(SC) prescales normalized f2 into fp8's sweet spot
        sbias = singles.tile([128, 1], FP32, tag="sbias", name="sbias")
        nc.any.memset(sbias[:], float(np.log(SC)))

        # resident fp8 GEMM operands: raw f1^T and prescaled-normalized f2^T
        n1t8 = singles.tile([128, KT, MC], FP8, tag="n1t8", name="n1t8")
        invn1T = singles.tile([128, MT], FP32, tag="invn1T", name="invn1T")
        dvals = singles.tile([128, MT], FP32, tag="dvals", name="dvals")
        spart = singles.tile([128, MT * NPAIR], FP32, tag="spart", name="spart")

        n2p = ctx.enter_context(tc.tile_pool(name="n2p", bufs=1))
        n2f8 = n2p.tile([128, KT, N], FP8, tag="n2f8", name="n2f8")
        stg = ctx.enter_context(tc.tile_pool(name="stg", bufs=2))
        pe0 = ctx.enter_context(tc.tile_pool(name="pe0", bufs=1))
        a_all = pe0.tile([128, MT, D], BF16, tag="a_all", name="a_all")
        n1ts = pe0.tile([128, KT, MC], BF16, tag="n1ts", name="n1ts")
        p0s = ctx.enter_context(tc.tile_pool(name="p0s", bufs=1))
        ss1 = p0s.tile([128, MT], FP32, tag="ss1", name="ss1")
        ss2 = p0s.tile([128, MT], FP32, tag="ss2", name="ss2")
        sssum = p0s.tile([128, MT], FP32, tag="sssum", name="sssum")
        draw = p0s.tile([128, MT], FP32, tag="draw", name="draw")

        # ---- hoisted loads, in consumption-priority order: the SDMA pool
        # round-robins queued work at packet granularity, so early bytes
        # delay pair-0 readiness 1:1.  f2tp is packed [pair][p][k][n] on the
        # host: each pair is one 3 MB DMA with 24 KB-contiguous reads.
        # bufs=2 staging backpressures the Pool FIFO into a 2-ahead stream.
        stages = []
        for pair in range(NPAIR):
            st = stg.tile([128, KT, PAIR], BF16, tag="stage", name="stage")
            stages.append(st)
        nc.gpsimd.dma_start(stages[0][:], f2tp[0])
        for k in range(KT):
            nc.gpsimd.dma_start(n1ts[:, k, :], f1t[k * 128 : (k + 1) * 128, :])
        for m in range(MT):
            nc.gpsimd.dma_start(a_all[:, m, :], f1n[m * 128 : (m + 1) * 128, :])
        for pair in range(1, NPAIR):
            nc.gpsimd.dma_start(stages[pair][:], f2tp[pair])

        def p0a_act():
            # emitted mid-pair-0 so these don't clog the ACT FIFO ahead of
            # pair-0's normalize chain; consumed only by the (later) Exps.
            nc.vector.tensor_copy(n1t8[:], n1ts[:])
            for m in range(MT):
                sqa = pe0.tile([128, D], BF16, tag="sqa", name="sqa", bufs=2)
                nc.scalar.activation(
                    sqa[:], a_all[:, m, :], AF.Square, accum_out=ss1[:, m : m + 1]
                )
            # invn1T = exp(-0.5*ln(ss1) + ln(1/(TEMP*SC)))
            t1 = p0s.tile([128, MT], FP32, tag="t1", name="t1")
            nc.scalar.activation(t1[:], ss1[:], AF.Ln)
            nc.scalar.activation(invn1T[:], t1[:], AF.Exp, scale=-0.5, bias=lbias[:])

        # ---- pair pipeline: normalize chunk, GEMM, fused exp/row-sum ----
        with tc.tile_pool(name="wk", bufs=2) as wp, tc.tile_pool(
            name="pss", bufs=2, space="PSUM"
        ) as pp, tc.tile_pool(name="psl", bufs=2, space="PSUM") as pl, tc.tile_pool(
            name="expp", bufs=3
        ) as ep:
            def do_pair(pair, mid=None):
                c0 = pair * PAIR
                st = stages[pair]
                sq = wp.tile([128, KT, PAIR], BF16, tag="sq", name="sq")
                for k in range(KT):
                    nc.vector.tensor_mul(sq[:, k, :], st[:, k, :], st[:, k, :])
                # per-f2-row sum of squares, broadcast to all 128 partitions
                ss = pp.tile([128, PAIR], FP32, tag="ss", name="ss")
                for k in range(KT):
                    for h in range(2):
                        nc.tensor.matmul(
                            ss[:, h * 512 : (h + 1) * 512],
                            ones[:],
                            sq[:, k, h * 512 : (h + 1) * 512],
                            start=(k == 0),
                            stop=(k == KT - 1),
                        )
                lntmp = wp.tile([128, PAIR], FP32, tag="lntmp", name="lntmp", bufs=1)
                nc.scalar.activation(lntmp[:], ss[:], AF.Ln)
                # invn2 = SC / ||f2_j||   (prescale keeps fp8 quanta small)
                invn2 = wp.tile([128, PAIR], BF16, tag="invn2", name="invn2")
                nc.scalar.activation(
                    invn2[:], lntmp[:], AF.Exp, scale=-0.5, bias=sbias[:]
                )
                # normalized+prescaled chunk into the resident fp8 tile
                for k in range(KT):
                    nc.vector.tensor_mul(
                        n2f8[:, k, c0 : c0 + PAIR], st[:, k, :], invn2[:]
                    )
                if mid is not None:
                    mid()
                # main GEMM (fp8 DoubleRow: k-chunk pairs) + fused exp/row-sum
                for m in range(MT):
                    pslog = pl.tile([128, PAIR], FP32, tag="pslog", name="pslog")
                    for j in range(KT // 2):
                        for h in range(2):
                            nc.tensor.matmul(
                                pslog[:, h * 512 : (h + 1) * 512],
                                n1t8[:, 2 * j : 2 * j + 2, m * 128 : (m + 1) * 128],
                                n2f8[
                                    :,
                                    2 * j : 2 * j + 2,
                                    c0 + h * 512 : c0 + (h + 1) * 512,
                                ],
                                start=(j == 0),
                                stop=(j == KT // 2 - 1),
                                perf_mode=mybir.MatmulPerfMode.DoubleRow,
                            )
                    eb = ep.tile([128, PAIR], BF16, tag="eb", name="eb")
                    col = m * NPAIR + pair
                    nc.scalar.activation(
                        eb[:],
                        pslog[:],
                        AF.Exp,
                        scale=invn1T[:, m : m + 1],
                        accum_out=spart[:, col : col + 1],
                    )

            do_pair(0, mid=p0a_act)
            for pair in range(1, NPAIR - 1):
                do_pair(pair)

            # ---- P0b: diagonal chain (emitted before the last pair so its
            # ACT/DVE work hides under the final matmul wave) ----
            # ab_all pulls double duty: first holds b (for ss2), then the
            # SWDGE accumulate adds a on top, giving a+b (for sssum).
            pab_ctx = ExitStack()
            pab = pab_ctx.enter_context(tc.tile_pool(name="pab", bufs=1))
            ab_all = pab.tile([128, MT, D], FP32, tag="ab_all", name="ab_all")
            for m in range(MT):
                nc.gpsimd.dma_start(
                    ab_all[:, m, :], f2dn[m * 128 : (m + 1) * 128, :]
                )
            for m in range(MT):
                sqb = ep.tile([128, D], BF16, tag="sqb", name="sqb", bufs=2)
                nc.scalar.activation(
                    sqb[:], ab_all[:, m, :], AF.Square, accum_out=ss2[:, m : m + 1]
                )
            for m in range(MT):
                nc.gpsimd.dma_start(
                    ab_all[:, m, :],
                    f1n[m * 128 : (m + 1) * 128, :],
                    accum_op=ALU.add,
                )
            for m in range(MT):
                sqc = ep.tile([128, D], BF16, tag="sqc", name="sqc", bufs=2)
                nc.scalar.activation(
                    sqc[:], ab_all[:, m, :], AF.Square, accum_out=sssum[:, m : m + 1]
                )
            t2 = p0s.tile([128, MT], FP32, tag="t2", name="t2")
            nc.scalar.activation(t2[:], ss2[:], AF.Ln)
            invn2d = p0s.tile([128, MT], FP32, tag="invn2d", name="invn2d")
            nc.scalar.activation(invn2d[:], t2[:], AF.Exp, scale=-0.5, bias=hbias[:])
            # draw = sssum - ss1 - ss2 = 2*<f1_i, f2_i>
            t4 = ep.tile([128, MT], FP32, tag="t4", name="t4", bufs=1)
            nc.vector.tensor_sub(t4[:], sssum[:], ss1[:])
            nc.vector.tensor_sub(draw[:], t4[:], ss2[:])
            # dvals = draw * invn1T * invn2d   (logit value on the diagonal)
            t3 = ep.tile([128, MT], FP32, tag="t3", name="t3", bufs=1)
            nc.vector.tensor_mul(t3[:], draw[:], invn1T[:])
            nc.vector.tensor_mul(dvals[:], t3[:], invn2d[:])

            do_pair(NPAIR - 1)

            # ---- finalize ----
            S = ep.tile([128, MT], FP32, tag="S", name="S", bufs=1)
            nc.vector.reduce_sum(
                S[:], spart[:].rearrange("p (m q) -> p m q", q=NPAIR), axis=AX.X
            )
            lse = ep.tile([128, MT], FP32, tag="lse", name="lse", bufs=1)
            nc.scalar.activation(lse[:], S[:], AF.Ln)
            res = ep.tile([128, MT], FP32, tag="res", name="res", bufs=1)
            nc.vector.tensor_sub(res[:], lse[:], dvals[:])
            nc.sync.dma_start(out[:, :], res[:])
            pab_ctx.close()

    _split_excess_waits(nc)
    return nc


def make_in_maps(f1: np.ndarray, f2: np.ndarray) -> list[dict[str, np.ndarray]]:
    f1 = np.ascontiguousarray(np.asarray(f1, dtype=np.float32))
    f2 = np.ascontiguousarray(np.asarray(f2, dtype=np.float32))
    assert f1.shape == (N, D) and f2.shape == (N, D)
    f2t = f2.T  # [D, N]
    # pack pair-major, partition-major: f2tp[q, p, k, n] = f2t[k*128+p, q*1024+n]
    f2tp = np.ascontiguousarray(
        f2t.reshape(KT, 128, NPAIR, PAIR).transpose(2, 1, 0, 3)
    )
    in_maps = []
    for c in range(NCORES):
        f1c = np.ascontiguousarray(f1[c * MC : (c + 1) * MC])
        in_maps.append(
            {
                "f1n": f1c,
                "f1t": np.ascontiguousarray(f1c.T),
                "f2tp": f2tp,
                "f2dn": np.ascontiguousarray(f2[c * MC : (c + 1) * MC]),
            }
        )
    return in_maps


def combine_outputs(outs: list[np.ndarray]) -> np.float32:
    total = 0.0
    for o in outs:
        total += float(np.sum(np.asarray(o, dtype=np.float64)))
    return np.float32(total / float(N))


def run(f1: np.ndarray, f2: np.ndarray, trace: bool = False):
    from concourse.bass_utils import run_bass_kernel_spmd

    nc = build_program()
    in_maps = make_in_maps(f1, f2)
    r = run_bass_kernel_spmd(nc, in_maps, core_ids=list(range(NCORES)), trace=trace)
    outs = [m["out"] for m in r.results]
    return combine_outputs(outs), r


def kernel(f1: np.ndarray, f2: np.ndarray) -> np.ndarray:
    loss, _ = run(f1, f2, trace=False)
    return loss


if __name__ == "__main__":
    f1 = np.random.randn(N, D).astype(np.float32)
    f2 = np.random.randn(N, D).astype(np.float32)
    print(kernel(f1, f2))
